# revision 1
# baseline (speedup 1.0000x reference)
"""Trainium2 Bass kernel for AutoRegressiveLSTMEncoder.

Strategy: pure data parallel over 8 NeuronCores (batch 32768 -> 4096/core).
All tensors live feature-on-partition / batch-on-free ("transposed") so every
matmul is lhsT.T @ rhs with K on partitions.

Key algebraic optimizations:
  - softmax(log(softplus(s)+eps)) == (softplus(s)+eps) / sum(softplus(s)+eps)
    -> no exp/log needed, and no max-subtraction (values are bounded).
  - The input-side term W_ih[:, :H] @ t_h + b_ih + b_hh is step-invariant:
    precompute once as G0 (saves 1/3 of the per-step FLOPs).
  - Per-step gates = G0 + W_ih[:, H:] @ e + W_hh @ h  (bf16 matmuls, fp32 acc).

The 32 LSTM steps run in a For_i hardware loop (16 iterations x 2 steps for
static ping-pong state addressing); per-step probs are written to the output
parity-major (p_all[2][16][A][B_local]) so the only dynamic address is the
loop counter itself. Host reassembles [B, D, A].
"""

import sys

sys.path.insert(0, "/opt/trn_rl_repo")

import numpy as np
import ml_dtypes
from contextlib import ExitStack

import concourse.bass as bass
import concourse.bacc as bacc
import concourse.tile as tile
from concourse import mybir

AF = mybir.ActivationFunctionType
DT = mybir.dt

# Problem dims (hardcoded per contest contract)
B, E, D, A, H = 32768, 300, 32, 64, 1024
G4 = 4 * H  # 4096
NCORES = 8
BL = B // NCORES  # 4096
NT = 512  # moving free-dim per matmul (one fp32 PSUM bank)
EPS = 1e-6
KXP = 384  # E=300 padded to 3*128


def build_nc(BL=BL, NB=None, nsteps=D, use_for_i=True):
    """Build the SPMD Bass program for one core handling BL batch elements."""
    if NB is None:
        NB = BL // NT
    assert BL == NB * NT and nsteps % 2 == 0
    NS2 = nsteps // 2

    nc = bacc.Bacc("TRN2", target_bir_lowering=False, debug=False)
    f32, bf = DT.float32, DT.bfloat16

    # ---- external inputs (host pre-tiled / pre-transposed / pre-cast) ----
    xT = nc.dram_tensor("xT", (3, 128, BL), bf, kind="ExternalInput")
    WxhT = nc.dram_tensor("WxhT", (3, 128, H), bf, kind="ExternalInput")
    bxh = nc.dram_tensor("bxh", (128, 8), f32, kind="ExternalInput")
    WihAT = nc.dram_tensor("WihAT", (8, 128, G4), bf, kind="ExternalInput")
    WbigT = nc.dram_tensor("WbigT", (A, G4), bf, kind="ExternalInput")
    WhhT = nc.dram_tensor("WhhT", (8, 128, G4), bf, kind="ExternalInput")
    bg = nc.dram_tensor("bg", (128, 32), f32, kind="ExternalInput")
    WhzT = nc.dram_tensor("WhzT", (8, 128, A), bf, kind="ExternalInput")
    bhz = nc.dram_tensor("bhz", (A, 1), f32, kind="ExternalInput")
    onesA = nc.dram_tensor("onesA", (A, 1), f32, kind="ExternalInput")
    ones1 = nc.dram_tensor("ones1", (1, 128), f32, kind="ExternalInput")

    # ---- output: parity-major probs ----
    p_all = nc.dram_tensor("p_all", (2, NS2, A, BL), f32, kind="ExternalOutput")

    # ---- internal DRAM scratch ----
    th_d = nc.dram_tensor("th_d", (8, 128, BL), bf, kind="Internal")
    G0_d = nc.dram_tensor("G0_d", (NB, 8, 128, 4 * NT), bf, kind="Internal")
    h_d = [nc.dram_tensor(f"h_d{i}", (8, 128, BL), bf, kind="Internal") for i in (0, 1)]
    p_d = [nc.dram_tensor(f"p_d{i}", (A, BL), bf, kind="Internal") for i in (0, 1)]
    c_d = [
        nc.dram_tensor(f"c_d{i}", (NB, 8, 128, NT), f32, kind="Internal")
        for i in (0, 1)
    ]

    with tile.TileContext(nc) as tc, ExitStack() as ctx:
        # ================= prologue =================
        with ExitStack() as pro:
            cpool = pro.enter_context(tc.tile_pool(name="pc", bufs=1))
            pspool = pro.enter_context(tc.tile_pool(name="pps", bufs=8, space="PSUM"))

            # zero-init state buffers (set 0)
            ztile = cpool.tile([128, NT], f32, tag="z32")
            nc.vector.memset(ztile[:], 0.0)
            zbf = cpool.tile([128, NT], bf, tag="zbf")
            nc.vector.tensor_copy(zbf[:], ztile[:])
            for r in range(8):
                for n in range(NB):
                    nc.sync.dma_start(c_d[0][n, r], ztile[:])
                    nc.sync.dma_start(h_d[0][r][:, n * NT : (n + 1) * NT], zbf[:])
            for n in range(NB):
                nc.sync.dma_start(p_d[0][:, n * NT : (n + 1) * NT], zbf[:A, :])

            # t_h = tanh(W_xh @ xT + b_xh)
            wxh = [cpool.tile([128, H], bf, tag=f"wxh{k}", name=f"wxh{k}") for k in range(3)]
            for k in range(3):
                nc.sync.dma_start(wxh[k][:], WxhT[k])
            bxh_t = cpool.tile([128, 8], f32, tag="bxh")
            nc.sync.dma_start(bxh_t[:], bxh[:])
            bg_t = cpool.tile([128, 32], f32, tag="bg")
            nc.sync.dma_start(bg_t[:], bg[:])

            xr_pool = pro.enter_context(tc.tile_pool(name="pxr", bufs=2))
            th_pool = pro.enter_context(tc.tile_pool(name="pth", bufs=2))
            for n in range(NB):
                xr = [xr_pool.tile([128, NT], bf, tag=f"xr{k}", name=f"xr{k}") for k in range(3)]
                for k in range(3):
                    nc.sync.dma_start(xr[k][:], xT[k][:, n * NT : (n + 1) * NT])
                for m in range(8):
                    ps = pspool.tile([128, NT], f32, tag="ps")
                    for k in range(3):
                        nc.tensor.matmul(
                            ps[:],
                            wxh[k][:, m * 128 : (m + 1) * 128],
                            xr[k][:],
                            start=(k == 0),
                            stop=(k == 2),
                        )
                    tht = th_pool.tile([128, NT], bf, tag="tht")
                    nc.scalar.activation(tht[:], ps[:], AF.Tanh, bias=bxh_t[:, m : m + 1])
                    nc.sync.dma_start(th_d[m][:, n * NT : (n + 1) * NT], tht[:])

            # G0 = W_ihA @ t_h + (b_ih + b_hh)   (bf16, pre-tiled by (n, r))
            wa_pool = pro.enter_context(tc.tile_pool(name="pwa", bufs=1))
            wa = [wa_pool.tile([128, G4], bf, tag=f"wa{k}", name=f"wa{k}") for k in range(8)]
            for k in range(8):
                nc.sync.dma_start(wa[k][:], WihAT[k])
            thr_pool = pro.enter_context(tc.tile_pool(name="pthr", bufs=2))
            g0_pool = pro.enter_context(tc.tile_pool(name="pg0", bufs=2))
            for n in range(NB):
                thr = [thr_pool.tile([128, NT], bf, tag=f"thr{k}", name=f"thr{k}") for k in range(8)]
                for k in range(8):
                    nc.sync.dma_start(thr[k][:], th_d[k][:, n * NT : (n + 1) * NT])
                for r in range(8):
                    g0t = g0_pool.tile([128, 4 * NT], bf, tag="g0t")
                    for gi in range(4):
                        m = gi * 8 + r
                        ps = pspool.tile([128, NT], f32, tag="ps")
                        for k in range(8):
                            nc.tensor.matmul(
                                ps[:],
                                wa[k][:, m * 128 : (m + 1) * 128],
                                thr[k][:],
                                start=(k == 0),
                                stop=(k == 7),
                            )
                        nc.scalar.activation(
                            g0t[:, gi * NT : (gi + 1) * NT],
                            ps[:],
                            AF.Identity,
                            bias=bg_t[:, m : m + 1],
                        )
                    nc.sync.dma_start(G0_d[n, r], g0t[:])

        # ================= resident weights =================
        wres = ctx.enter_context(tc.tile_pool(name="wres", bufs=1))
        wh = [wres.tile([128, G4], bf, tag=f"wh{k}", name=f"wh{k}") for k in range(8)]
        for k in range(8):
            nc.sync.dma_start(wh[k][:], WhhT[k])
        wb_t = wres.tile([A, G4], bf, tag="wbig")
        nc.sync.dma_start(wb_t[:], WbigT[:])
        wz = [wres.tile([128, A], bf, tag=f"wz{k}", name=f"wz{k}") for k in range(8)]
        for k in range(8):
            nc.sync.dma_start(wz[k][:], WhzT[k])
        ones_t = wres.tile([A, 1], f32, tag="onesA")
        nc.sync.dma_start(ones_t[:], onesA[:])
        ones1_t = wres.tile([1, 128], f32, tag="ones1")
        nc.sync.dma_start(ones1_t[:], ones1[:])
        bhz_t = wres.tile([A, 1], f32, tag="bhz")
        nc.sync.dma_start(bhz_t[:], bhz[:])
        eps_t = wres.tile([A, 1], f32, tag="eps")
        nc.vector.memset(eps_t[:], EPS)

        # ================= main loop pools =================
        psum = ctx.enter_context(tc.tile_pool(name="psum", bufs=8, space="PSUM"))
        rhs_p = ctx.enter_context(tc.tile_pool(name="rhs", bufs=2))
        g0r_p = ctx.enter_context(tc.tile_pool(name="g0r", bufs=2))
        cin_p = ctx.enter_context(tc.tile_pool(name="cin", bufs=2))
        cell_p = ctx.enter_context(tc.tile_pool(name="cell", bufs=2))
        # gate tiles cycle fast; 2 bufs each via tag is enough
        hn_p = ctx.enter_context(tc.tile_pool(name="hn", bufs=1))
        zp_p = ctx.enter_context(tc.tile_pool(name="zp", bufs=2))
        ev_p = ctx.enter_context(tc.tile_pool(name="ev", bufs=2))

        def lstm_step(rb, wb, parity, jv):
            """One LSTM step: read state[rb], write state[wb], probs->p_all[parity][jv]."""
            for n in range(NB):
                sl = slice(n * NT, (n + 1) * NT)
                pr = rhs_p.tile([A, NT], bf, tag="pr")
                nc.sync.dma_start(pr[:], p_d[rb][:, sl])
                hr = [rhs_p.tile([128, NT], bf, tag=f"hr{k}", name=f"hr{k}") for k in range(8)]
                for k in range(8):
                    nc.sync.dma_start(hr[k][:], h_d[rb][k][:, sl])
                hnew = []
                for r in range(8):
                    g0t = g0r_p.tile([128, 4 * NT], bf, tag="g0t")
                    nc.sync.dma_start(g0t[:], G0_d[n, r])
                    cin = cin_p.tile([128, NT], f32, tag="cin")
                    nc.sync.dma_start(cin[:], c_d[rb][n, r])
                    gps = []
                    for gi in range(4):
                        m = gi * 8 + r
                        ps = psum.tile([128, NT], f32, tag="ps")
                        for k in range(8):
                            nc.tensor.matmul(
                                ps[:],
                                wh[k][:, m * 128 : (m + 1) * 128],
                                hr[k][:],
                                start=(k == 0),
                                stop=False,
                            )
                        nc.tensor.matmul(
                            ps[:],
                            wb_t[:, m * 128 : (m + 1) * 128],
                            pr[:],
                            start=False,
                            stop=True,
                        )
                        # add G0 and apply gate nonlinearity in-place in PSUM
                        nc.vector.tensor_tensor(
                            ps[:], ps[:], g0t[:, gi * NT : (gi + 1) * NT],
                            mybir.AluOpType.add,
                        )
                        # evict gate activation to SBUF immediately: frees the
                        # PSUM bank after one ACT and keeps the cell math in
                        # SBUF (DVE fast path, no PSUM-read limits)
                        gsb = cell_p.tile(
                            [128, NT], f32, tag=f"gate{gi}", name=f"gate{gi}"
                        )
                        nc.scalar.activation(
                            gsb[:], ps[:], AF.Tanh if gi == 2 else AF.Sigmoid
                        )
                        gps.append(gsb)
                    i_sb, f_sb, g_sb, o_sb = gps
                    # c' = f*c + i*g ; h = o*tanh(c')
                    ig_sb = cell_p.tile([128, NT], f32, tag="igsb")
                    nc.vector.tensor_tensor(ig_sb[:], g_sb[:], i_sb[:], mybir.AluOpType.mult)
                    nc.vector.tensor_tensor(f_sb[:], f_sb[:], cin[:], mybir.AluOpType.mult)
                    cnew = cell_p.tile([128, NT], f32, tag="cnew")
                    nc.vector.tensor_tensor(cnew[:], f_sb[:], ig_sb[:], mybir.AluOpType.add)
                    nc.sync.dma_start(c_d[wb][n, r], cnew[:])
                    tht = cell_p.tile([128, NT], f32, tag="tht")
                    nc.scalar.activation(tht[:], cnew[:], AF.Tanh)
                    hbf = hn_p.tile([128, NT], bf, tag=f"hn{r}", name=f"hn{r}")
                    nc.vector.tensor_tensor(hbf[:], o_sb[:], tht[:], mybir.AluOpType.mult)
                    nc.sync.dma_start(h_d[wb][r][:, sl], hbf[:])
                    hnew.append(hbf)
                # z/p/e phase
                zps = psum.tile([A, NT], f32, tag="ps")
                for k in range(8):
                    nc.tensor.matmul(
                        zps[:], wz[k][:], hnew[k][:], start=(k == 0), stop=(k == 7)
                    )
                u = zp_p.tile([A, NT], f32, tag="u")
                nc.scalar.activation(u[:], zps[:], AF.Exp, bias=bhz_t[:])
                q = zp_p.tile([A, NT], f32, tag="q")
                nc.scalar.activation(q[:], u[:], AF.Ln, bias=ones_t[:])
                q2 = zp_p.tile([A, NT], f32, tag="q2")
                nc.scalar.activation(q2[:], q[:], AF.Identity, bias=eps_t[:])
                sps = psum.tile([1, NT], f32, tag="ps")
                nc.tensor.matmul(sps[:], ones_t[:], q2[:], start=True, stop=True)
                rec = zp_p.tile([1, NT], f32, tag="rec")
                nc.vector.reciprocal(rec[:], sps[:])
                rbc = psum.tile([128, NT], f32, tag="ps")
                nc.tensor.matmul(rbc[:], ones1_t[:], rec[:], start=True, stop=True)
                pt = zp_p.tile([A, NT], f32, tag="pt")
                nc.vector.tensor_tensor(pt[:], q2[:], rbc[:A, :], mybir.AluOpType.mult)
                if isinstance(jv, int):
                    nc.sync.dma_start(p_all[parity, jv][:, sl], pt[:])
                else:
                    nc.sync.dma_start(p_all[parity][bass.ds(jv, 1)][:, :, sl], pt[:])
                pbf = zp_p.tile([A, NT], bf, tag="pbf")
                nc.vector.tensor_copy(pbf[:], pt[:])
                nc.sync.dma_start(p_d[wb][:, sl], pbf[:])

        if use_for_i:
            with tc.For_i(0, NS2, 1) as j:
                lstm_step(0, 1, 0, j)
                lstm_step(1, 0, 1, j)
        else:
            for t in range(nsteps):
                lstm_step(t % 2, (t + 1) % 2, t % 2, t // 2)

    nc.compile()
    return nc


# ---------------- host-side wrapper ----------------


def _prep_weights(W_xh, b_xh, W_ih, W_hh, b_ih, b_hh, W_hz, b_hz, W_emb):
    bf = ml_dtypes.bfloat16
    f32 = np.float32
    d = {}
    wxh = np.zeros((KXP, H), f32)
    wxh[:E] = np.asarray(W_xh, f32).T
    d["WxhT"] = np.ascontiguousarray(wxh.reshape(3, 128, H)).astype(bf)
    d["bxh"] = np.ascontiguousarray(np.asarray(b_xh, f32).reshape(8, 128).T)
    wih = np.asarray(W_ih, f32)
    d["WihAT"] = np.ascontiguousarray(wih[:, :H].T.reshape(8, 128, G4)).astype(bf)
    wbig = wih[:, H:].astype(np.float64) @ np.asarray(W_emb, np.float64)
    d["WbigT"] = np.ascontiguousarray(wbig.T.astype(np.float32)).astype(bf)
    d["WhhT"] = np.ascontiguousarray(np.asarray(W_hh, f32).T.reshape(8, 128, G4)).astype(bf)
    d["bg"] = np.ascontiguousarray(
        (np.asarray(b_ih, f32) + np.asarray(b_hh, f32)).reshape(32, 128).T
    )
    d["WhzT"] = np.ascontiguousarray(np.asarray(W_hz, f32).T.reshape(8, 128, A)).astype(bf)
    d["bhz"] = np.ascontiguousarray(np.asarray(b_hz, f32).reshape(A, 1))
    d["onesA"] = np.ones((A, 1), f32)
    d["ones1"] = np.ones((1, 128), f32)
    return d


def _prep_x(x_shard):
    bf = ml_dtypes.bfloat16
    xt = np.zeros((KXP, x_shard.shape[0]), np.float32)
    xt[:E] = np.asarray(x_shard, np.float32).T
    return np.ascontiguousarray(xt.reshape(3, 128, -1)).astype(bf)


def kernel(input_x, W_xh, b_xh, W_ih, W_hh, b_ih, b_hh, W_hz, b_hz, W_emb):
    from concourse.bass_utils import run_bass_kernel_spmd

    wd = _prep_weights(W_xh, b_xh, W_ih, W_hh, b_ih, b_hh, W_hz, b_hz, W_emb)
    x = np.asarray(input_x, np.float32)
    in_maps = []
    for c in range(NCORES):
        m = dict(wd)
        m["xT"] = _prep_x(x[c * BL : (c + 1) * BL])
        in_maps.append(m)

    nc = build_nc()
    res = run_bass_kernel_spmd(nc, in_maps, list(range(NCORES)))
    global LAST_RESULT
    LAST_RESULT = res

    out = np.empty((B, D, A), np.float32)
    for c in range(NCORES):
        pa = res.results[c]["p_all"]  # [2, 16, A, BL]
        p = np.empty((D, A, BL), np.float32)
        p[0::2] = pa[0]
        p[1::2] = pa[1]
        out[c * BL : (c + 1) * BL] = p.transpose(2, 0, 1)
    return out, out



# revision 2
# speedup vs baseline: 1.0428x; 1.0428x over previous
"""Trainium2 Bass kernel for AutoRegressiveLSTMEncoder.

Strategy: pure data parallel over 8 NeuronCores (batch 32768 -> 4096/core).
All tensors live feature-on-partition / batch-on-free ("transposed") so every
matmul is lhsT.T @ rhs with K on partitions.

Key algebraic optimizations (kept from v1):
  - softmax(log(softplus(s)+eps)) == (softplus(s)+eps) / sum(softplus(s)+eps)
    -> no exp/log softmax needed, no max-subtraction (values are bounded).
    eps is dropped entirely: softplus(z) >= 0.3 for the z ranges here, so
    eps=1e-6 perturbs p by ~1e-6 relative (tolerance is 2e-2).
  - The input-side term W_ih[:, :H] @ t_h + b_ih + b_hh is step-invariant:
    precompute once per batch as G0.
  - W_ih[:, H:] @ W_emb folded host-side into Wbig [4H, A]; the embedding e
    never materializes, p (64-dim) feeds the gates directly.

v2 structural change vs v1: the LSTM state (h, c, p) never round-trips DRAM.
The per-core batch (4096) is processed as 8 blocks of 512 columns; each block
runs all 32 steps with state resident in SBUF. G0 for the block (bf16
[4096 x 512]) is DMA'd into SBUF once per block. The only per-step DMA is the
64x512 prob tile store. A For_i hardware loop runs the 8 blocks; the 32 steps
are fully unrolled inside the body so the tensor engine sees an uninterrupted
stream of matmuls (per step: 256 W_hh + 32 Wbig + 8 W_hz + 1 sum = 297
matmuls, ~63us; DVE/ACT trail under that).

Output: probs are written bf16 as p_out[step][block][A][512]; host converts
to fp32 and reassembles [B, D, A].
"""

import sys

sys.path.insert(0, "/opt/trn_rl_repo")

import numpy as np
import ml_dtypes
from contextlib import ExitStack

import concourse.bass as bass
import concourse.bacc as bacc
import concourse.tile as tile
from concourse import mybir

AF = mybir.ActivationFunctionType
DT = mybir.dt
ET = mybir.EngineType

# Problem dims (hardcoded per contest contract)
B, E, D, A, H = 32768, 300, 32, 64, 1024
G4 = 4 * H  # 4096
NCORES = 8
BL = B // NCORES  # 4096
NT = 512  # block width = one fp32 PSUM bank
NB = BL // NT  # 8 blocks per core
KXP = 384  # E=300 padded to 3*128


def build_nc(nsteps=D, use_for_i=True, nblocks=NB):
    """Build the SPMD Bass program for one core handling BL batch elements."""
    nc = bacc.Bacc("TRN2", target_bir_lowering=False, debug=False)
    f32, bf = DT.float32, DT.bfloat16

    # ---- external inputs (host pre-tiled / pre-transposed / pre-cast) ----
    xT = nc.dram_tensor("xT", (3, 128, BL), bf, kind="ExternalInput")
    WxhT = nc.dram_tensor("WxhT", (3, 128, H), bf, kind="ExternalInput")
    bxh = nc.dram_tensor("bxh", (128, 8), f32, kind="ExternalInput")
    WihAT = nc.dram_tensor("WihAT", (8, 128, G4), bf, kind="ExternalInput")
    WbigT = nc.dram_tensor("WbigT", (A, G4), bf, kind="ExternalInput")
    WhhT = nc.dram_tensor("WhhT", (8, 128, G4), bf, kind="ExternalInput")
    bg = nc.dram_tensor("bg", (128, 32), f32, kind="ExternalInput")
    WhzT = nc.dram_tensor("WhzT", (8, 128, A), bf, kind="ExternalInput")
    bhz = nc.dram_tensor("bhz", (A, 1), f32, kind="ExternalInput")

    # ---- output: step-major bf16 probs ----
    p_out = nc.dram_tensor("p_out", (nsteps, nblocks, A, NT), bf, kind="ExternalOutput")

    # ---- internal DRAM scratch ----
    th_d = nc.dram_tensor("th_d", (8, 128, BL), bf, kind="Internal")
    # r-major so the main loop can index [r][dynamic block]
    G0_d = nc.dram_tensor("G0_d", (8, NB, 128, 4 * NT), bf, kind="Internal")

    with tile.TileContext(nc) as tc, ExitStack() as ctx:
        # ================= prologue: t_h and G0 =================
        with ExitStack() as pro:
            cpool = pro.enter_context(tc.tile_pool(name="pc", bufs=1))
            pspool = pro.enter_context(tc.tile_pool(name="pps", bufs=8, space="PSUM"))

            # t_h = tanh(W_xh @ xT + b_xh)
            wxh = [cpool.tile([128, H], bf, tag=f"wxh{k}", name=f"wxh{k}") for k in range(3)]
            for k in range(3):
                nc.sync.dma_start(wxh[k][:], WxhT[k])
            bxh_t = cpool.tile([128, 8], f32, tag="bxh")
            nc.sync.dma_start(bxh_t[:], bxh[:])
            bg_t = cpool.tile([128, 32], f32, tag="bg")
            nc.sync.dma_start(bg_t[:], bg[:])

            xr_pool = pro.enter_context(tc.tile_pool(name="pxr", bufs=2))
            th_pool = pro.enter_context(tc.tile_pool(name="pth", bufs=2))
            for n in range(NB):
                xr = [xr_pool.tile([128, NT], bf, tag=f"xr{k}", name=f"xr{k}") for k in range(3)]
                for k in range(3):
                    nc.sync.dma_start(xr[k][:], xT[k][:, n * NT : (n + 1) * NT])
                for m in range(8):
                    ps = pspool.tile([128, NT], f32, tag="ps")
                    for k in range(3):
                        nc.tensor.matmul(
                            ps[:],
                            wxh[k][:, m * 128 : (m + 1) * 128],
                            xr[k][:],
                            start=(k == 0),
                            stop=(k == 2),
                        )
                    tht = th_pool.tile([128, NT], bf, tag="tht")
                    nc.scalar.activation(tht[:], ps[:], AF.Tanh, bias=bxh_t[:, m : m + 1])
                    nc.sync.dma_start(th_d[m][:, n * NT : (n + 1) * NT], tht[:])

            # G0 = W_ihA @ t_h + (b_ih + b_hh)   (bf16, tiled [r][n][128, 4*NT])
            wa_pool = pro.enter_context(tc.tile_pool(name="pwa", bufs=1))
            wa = [wa_pool.tile([128, G4], bf, tag=f"wa{k}", name=f"wa{k}") for k in range(8)]
            for k in range(8):
                nc.sync.dma_start(wa[k][:], WihAT[k])
            thr_pool = pro.enter_context(tc.tile_pool(name="pthr", bufs=2))
            g0_pool = pro.enter_context(tc.tile_pool(name="pg0", bufs=2))
            for n in range(NB):
                thr = [thr_pool.tile([128, NT], bf, tag=f"thr{k}", name=f"thr{k}") for k in range(8)]
                for k in range(8):
                    nc.sync.dma_start(thr[k][:], th_d[k][:, n * NT : (n + 1) * NT])
                for r in range(8):
                    g0t = g0_pool.tile([128, 4 * NT], bf, tag="g0t")
                    for gi in range(4):
                        m = gi * 8 + r
                        ps = pspool.tile([128, NT], f32, tag="ps")
                        for k in range(8):
                            nc.tensor.matmul(
                                ps[:],
                                wa[k][:, m * 128 : (m + 1) * 128],
                                thr[k][:],
                                start=(k == 0),
                                stop=(k == 7),
                            )
                        nc.scalar.activation(
                            g0t[:, gi * NT : (gi + 1) * NT],
                            ps[:],
                            AF.Identity,
                            bias=bg_t[:, m : m + 1],
                        )
                    nc.sync.dma_start(G0_d[r, n], g0t[:])

        # ================= resident weights =================
        wres = ctx.enter_context(tc.tile_pool(name="wres", bufs=1))
        wh = [wres.tile([128, G4], bf, tag=f"wh{k}", name=f"wh{k}") for k in range(8)]
        for k in range(8):
            nc.sync.dma_start(wh[k][:], WhhT[k])
        wb_t = wres.tile([A, G4], bf, tag="wbig")
        nc.sync.dma_start(wb_t[:], WbigT[:])
        wz = [wres.tile([128, A], bf, tag=f"wz{k}", name=f"wz{k}") for k in range(8)]
        for k in range(8):
            nc.sync.dma_start(wz[k][:], WhzT[k])
        bhz_t = wres.tile([A, 1], f32, tag="bhz")
        nc.sync.dma_start(bhz_t[:], bhz[:])
        one_b = wres.tile([A, 1], f32, tag="oneb")
        nc.vector.memset(one_b[:], 1.0)
        # all-ones [A, A] lhsT: one matmul = column-sum broadcast to A partitions
        onesbc = wres.tile([A, A], bf, tag="onesbc")
        nc.vector.memset(onesbc[:], 1.0)

        # ================= main loop pools =================
        psg = ctx.enter_context(tc.tile_pool(name="psg", bufs=6, space="PSUM"))
        psz = ctx.enter_context(tc.tile_pool(name="psz", bufs=1, space="PSUM"))
        g0r_p = ctx.enter_context(tc.tile_pool(name="g0r", bufs=1))
        h_p = ctx.enter_context(tc.tile_pool(name="hp", bufs=2))
        c_p = ctx.enter_context(tc.tile_pool(name="cp", bufs=1))
        gt_p = ctx.enter_context(tc.tile_pool(name="gt", bufs=2))
        cw_p = ctx.enter_context(tc.tile_pool(name="cw", bufs=2))
        z_p = ctx.enter_context(tc.tile_pool(name="zp", bufs=2))
        pp_p = ctx.enter_context(tc.tile_pool(name="pp", bufs=2))

        def block_body(nb_iv):
            """All nsteps LSTM steps for one 512-column batch block."""
            g0res = [
                g0r_p.tile([128, 4 * NT], bf, tag=f"g0r{r}", name=f"g0r{r}")
                for r in range(8)
            ]
            for r in range(8):
                if isinstance(nb_iv, int):
                    nc.sync.dma_start(g0res[r][:], G0_d[r, nb_iv])
                else:
                    nc.sync.dma_start(g0res[r][:], G0_d[r][bass.ds(nb_iv, 1)])

            ctiles = [None] * 8
            hprev = [None] * 8
            pprev = None

            def z_phase(t, hcur):
                """softplus(Whz@h + bhz) normalized -> p (bf16); store to p_out."""
                zps = psz.tile([A, NT], f32, tag="zps", name="zps")
                for k in range(8):
                    nc.tensor.matmul(
                        zps[:], wz[k][:], hcur[k][:], start=(k == 0), stop=(k == 7)
                    )
                u = z_p.tile([A, NT], f32, tag="u")
                nc.scalar.activation(u[:], zps[:], AF.Exp, bias=bhz_t[:])
                q2 = z_p.tile([A, NT], bf, tag="q2")
                nc.scalar.activation(q2[:], u[:], AF.Ln, bias=one_b[:])
                sb = psz.tile([A, NT], f32, tag="sb", name="sb")
                nc.tensor.matmul(sb[:], onesbc[:], q2[:], start=True, stop=True)
                rec = z_p.tile([A, NT], f32, tag="rec")
                nc.vector.reciprocal(rec[:], sb[:])
                pnew = pp_p.tile([A, NT], bf, tag="p", name="pnew")
                nc.vector.tensor_tensor(pnew[:], q2[:], rec[:], mybir.AluOpType.mult)
                if isinstance(nb_iv, int):
                    nc.sync.dma_start(p_out[t, nb_iv], pnew[:])
                else:
                    nc.sync.dma_start(p_out[t][bass.ds(nb_iv, 1)], pnew[:])
                return pnew

            # ---- step 0: h = c = p = 0, so gates = act(G0) straight from SBUF
            hcur = []
            for r in range(8):
                i_sb = gt_p.tile([128, NT], f32, tag="g0", name="i_sb")
                nc.scalar.activation(i_sb[:], g0res[r][:, 0:NT], AF.Sigmoid)
                g_sb = gt_p.tile([128, NT], f32, tag="g2", name="g_sb")
                nc.scalar.activation(g_sb[:], g0res[r][:, 2 * NT : 3 * NT], AF.Tanh)
                o_sb = gt_p.tile([128, NT], f32, tag="g3", name="o_sb")
                nc.scalar.activation(o_sb[:], g0res[r][:, 3 * NT : 4 * NT], AF.Sigmoid)
                c_r = c_p.tile([128, NT], f32, tag=f"c{r}", name=f"c{r}")
                nc.vector.tensor_tensor(c_r[:], i_sb[:], g_sb[:], mybir.AluOpType.mult)
                tht = cw_p.tile([128, NT], f32, tag="tht")
                nc.scalar.activation(tht[:], c_r[:], AF.Tanh)
                h_r = h_p.tile([128, NT], bf, tag=f"h{r}", name=f"h{r}")
                nc.vector.tensor_tensor(h_r[:], o_sb[:], tht[:], mybir.AluOpType.mult)
                ctiles[r] = c_r
                hcur.append(h_r)
            pprev = z_phase(0, hcur)
            hprev = hcur

            # ---- steps 1..nsteps-1
            for t in range(1, nsteps):
                hcur = []
                for r in range(8):
                    gates = []
                    for gi in range(4):
                        m = gi * 8 + r
                        ps = psg.tile([128, NT], f32, tag="ps", name="ps")
                        for k in range(8):
                            nc.tensor.matmul(
                                ps[:],
                                wh[k][:, m * 128 : (m + 1) * 128],
                                hprev[k][:],
                                start=(k == 0),
                                stop=False,
                            )
                        nc.tensor.matmul(
                            ps[:],
                            wb_t[:, m * 128 : (m + 1) * 128],
                            pprev[:],
                            start=False,
                            stop=True,
                        )
                        nc.vector.tensor_tensor(
                            ps[:], ps[:], g0res[r][:, gi * NT : (gi + 1) * NT],
                            mybir.AluOpType.add,
                        )
                        gsb = gt_p.tile(
                            [128, NT], f32, tag=f"g{gi}", name=f"gate{gi}"
                        )
                        nc.scalar.activation(
                            gsb[:], ps[:], AF.Tanh if gi == 2 else AF.Sigmoid
                        )
                        gates.append(gsb)
                    i_sb, f_sb, g_sb, o_sb = gates
                    # c' = f*c + i*g (c updated in place); h = o*tanh(c')
                    ig = cw_p.tile([128, NT], f32, tag="ig")
                    nc.vector.tensor_tensor(ig[:], g_sb[:], i_sb[:], mybir.AluOpType.mult)
                    nc.vector.tensor_tensor(
                        f_sb[:], f_sb[:], ctiles[r][:], mybir.AluOpType.mult
                    )
                    nc.vector.tensor_tensor(
                        ctiles[r][:], f_sb[:], ig[:], mybir.AluOpType.add
                    )
                    tht = cw_p.tile([128, NT], f32, tag="tht")
                    nc.scalar.activation(tht[:], ctiles[r][:], AF.Tanh)
                    h_r = h_p.tile([128, NT], bf, tag=f"h{r}", name=f"h{r}")
                    nc.vector.tensor_tensor(h_r[:], o_sb[:], tht[:], mybir.AluOpType.mult)
                    hcur.append(h_r)
                pprev = z_phase(t, hcur)
                hprev = hcur

        if use_for_i:
            with tc.For_i(
                0,
                nblocks,
                1,
                hint_engines=(ET.PE, ET.Activation, ET.DVE),
            ) as nb_iv:
                block_body(nb_iv)
        else:
            for nbi in range(nblocks):
                block_body(nbi)

    nc.compile()
    return nc


# ---------------- host-side wrapper ----------------


def _prep_weights(W_xh, b_xh, W_ih, W_hh, b_ih, b_hh, W_hz, b_hz, W_emb):
    bf = ml_dtypes.bfloat16
    f32 = np.float32
    d = {}
    wxh = np.zeros((KXP, H), f32)
    wxh[:E] = np.asarray(W_xh, f32).T
    d["WxhT"] = np.ascontiguousarray(wxh.reshape(3, 128, H)).astype(bf)
    d["bxh"] = np.ascontiguousarray(np.asarray(b_xh, f32).reshape(8, 128).T)
    wih = np.asarray(W_ih, f32)
    d["WihAT"] = np.ascontiguousarray(wih[:, :H].T.reshape(8, 128, G4)).astype(bf)
    wbig = wih[:, H:].astype(np.float64) @ np.asarray(W_emb, np.float64)
    d["WbigT"] = np.ascontiguousarray(wbig.T.astype(np.float32)).astype(bf)
    d["WhhT"] = np.ascontiguousarray(np.asarray(W_hh, f32).T.reshape(8, 128, G4)).astype(bf)
    d["bg"] = np.ascontiguousarray(
        (np.asarray(b_ih, f32) + np.asarray(b_hh, f32)).reshape(32, 128).T
    )
    d["WhzT"] = np.ascontiguousarray(np.asarray(W_hz, f32).T.reshape(8, 128, A)).astype(bf)
    d["bhz"] = np.ascontiguousarray(np.asarray(b_hz, f32).reshape(A, 1))
    return d


def _prep_x(x_shard):
    bf = ml_dtypes.bfloat16
    xt = np.zeros((KXP, x_shard.shape[0]), np.float32)
    xt[:E] = np.asarray(x_shard, np.float32).T
    return np.ascontiguousarray(xt.reshape(3, 128, -1)).astype(bf)


def kernel(input_x, W_xh, b_xh, W_ih, W_hh, b_ih, b_hh, W_hz, b_hz, W_emb):
    from concourse.bass_utils import run_bass_kernel_spmd

    wd = _prep_weights(W_xh, b_xh, W_ih, W_hh, b_ih, b_hh, W_hz, b_hz, W_emb)
    x = np.asarray(input_x, np.float32)
    in_maps = []
    for c in range(NCORES):
        m = dict(wd)
        m["xT"] = _prep_x(x[c * BL : (c + 1) * BL])
        in_maps.append(m)

    nc = build_nc()
    res = run_bass_kernel_spmd(nc, in_maps, list(range(NCORES)))
    global LAST_RESULT
    LAST_RESULT = res

    out = np.empty((B, D, A), np.float32)
    for c in range(NCORES):
        pa = np.asarray(res.results[c]["p_out"], dtype=np.float32)  # [D, NB, A, NT]
        # [D, NB, A, NT] -> [NB, NT, D, A] -> [BL, D, A]
        out[c * BL : (c + 1) * BL] = pa.transpose(1, 3, 0, 2).reshape(BL, D, A)
    return out, out


# revision 13
# speedup vs baseline: 1.6669x; 1.5984x over previous
"""Trainium2 Bass kernel for AutoRegressiveLSTMEncoder.

Strategy: pure data parallel over 8 NeuronCores (batch 32768 -> 4096/core).
All tensors live feature-on-partition / batch-on-free ("transposed") so every
matmul is lhsT.T @ rhs with K on partitions.

Algebraic optimizations:
  - softmax(log(softplus(s)+eps)) == softplus(s) / sum(softplus(s)) (eps is
    negligible against softplus >= 0.3 here; tolerance is 2e-2).
  - W_ih[:, :H] @ t_h + b_ih + b_hh is step-invariant: precomputed as G0.
  - W_ih[:, H:] @ W_emb folded host-side into Wbig [4H, A]; p feeds gates
    directly.

Structure (v2): per-core batch processed as 8 blocks of 512 columns; each
block runs all 32 steps with LSTM state (h, c, p) resident in SBUF, G0
resident per block. Only per-step DMA is the 64x512 bf16 prob store.

v3: the recurrent matmuls run in fp8 e4m3 with DoubleRow perf mode
(0.5 PE cycles/row, 4x fewer PE cycles than bf16): W_hh and W_hz weights are
fp8, h state is stored fp8 in k-paired [128, 2, 512] tiles. Wbig@p stays
bf16. Gate PSUM is paired (i,f) and (g,o) in [128, 1024] 2-bank tiles so the
G0 add and the (i,f) sigmoid run as single wide instructions. Cell-state
elementwise math runs on the Pool engine to keep DVE under the PE/ACT cap.
"""

import sys

sys.path.insert(0, "/opt/trn_rl_repo")

import numpy as np
import ml_dtypes
from contextlib import ExitStack

import concourse.bass as bass
import concourse.bacc as bacc
import concourse.tile as tile
from concourse import mybir

AF = mybir.ActivationFunctionType
DT = mybir.dt
ET = mybir.EngineType
DR = mybir.MatmulPerfMode.DoubleRow

# Problem dims (hardcoded per contest contract)
B, E, D, A, H = 32768, 300, 32, 64, 1024
G4 = 4 * H  # 4096
NCORES = 8
BL = B // NCORES  # 4096
NT = 512  # block width = one fp32 PSUM bank
NB = BL // NT  # 8 blocks per core
KXP = 384  # E=300 padded to 3*128


def build_nc(nsteps=D, use_for_i=True, nblocks=NB):
    """Build the SPMD Bass program for one core handling BL batch elements."""
    nc = bacc.Bacc("TRN2", target_bir_lowering=False, debug=False)
    f32, bf, f8 = DT.float32, DT.bfloat16, DT.float8e4

    # ---- external inputs (host pre-tiled / pre-transposed / pre-cast) ----
    xT = nc.dram_tensor("xT", (3, 128, BL), bf, kind="ExternalInput")
    WxhT = nc.dram_tensor("WxhT", (3, 128, H), bf, kind="ExternalInput")
    bxh = nc.dram_tensor("bxh", (128, 8), f32, kind="ExternalInput")
    WihAT = nc.dram_tensor("WihAT", (8, 128, G4), bf, kind="ExternalInput")
    WbigT = nc.dram_tensor("WbigT", (A, G4), bf, kind="ExternalInput")
    WhhP = nc.dram_tensor("WhhP", (4, 128, 2, G4), f8, kind="ExternalInput")
    bg = nc.dram_tensor("bg", (128, 32), f32, kind="ExternalInput")
    WhzP = nc.dram_tensor("WhzP", (4, 128, 2, A), f8, kind="ExternalInput")
    bhz = nc.dram_tensor("bhz", (A, 1), f32, kind="ExternalInput")
    eyeT = nc.dram_tensor("eyeT", (128, 128), bf, kind="ExternalInput")

    # ---- output: step-major bf16 probs ----
    p_out = nc.dram_tensor("p_out", (nsteps, nblocks, A, NT), bf, kind="ExternalOutput")

    # ---- internal DRAM scratch ----
    th_d = nc.dram_tensor("th_d", (8, 128, BL), bf, kind="Internal")
    # r-major so the main loop can index [r][dynamic block]
    G0_d = nc.dram_tensor("G0_d", (8, NB, 128, 4 * NT), bf, kind="Internal")

    with tile.TileContext(nc) as tc, ExitStack() as ctx:
        # ================= prologue: t_h and G0 =================
        with ExitStack() as pro:
            cpool = pro.enter_context(tc.tile_pool(name="pc", bufs=1))
            pspool = pro.enter_context(tc.tile_pool(name="pps", bufs=8, space="PSUM"))

            # t_h = tanh(W_xh @ xT + b_xh)
            wxh = [cpool.tile([128, H], bf, tag=f"wxh{k}", name=f"wxh{k}") for k in range(3)]
            for k in range(3):
                nc.sync.dma_start(wxh[k][:], WxhT[k])
            bxh_t = cpool.tile([128, 8], f32, tag="bxh")
            nc.sync.dma_start(bxh_t[:], bxh[:])
            bg_t = cpool.tile([128, 32], f32, tag="bg")
            nc.sync.dma_start(bg_t[:], bg[:])

            xr_pool = pro.enter_context(tc.tile_pool(name="pxr", bufs=2))
            th_pool = pro.enter_context(tc.tile_pool(name="pth", bufs=2))
            for n in range(NB):
                xr = [xr_pool.tile([128, NT], bf, tag=f"xr{k}", name=f"xr{k}") for k in range(3)]
                for k in range(3):
                    nc.sync.dma_start(xr[k][:], xT[k][:, n * NT : (n + 1) * NT])
                for m in range(8):
                    ps = pspool.tile([128, NT], f32, tag="ps")
                    for k in range(3):
                        nc.tensor.matmul(
                            ps[:],
                            wxh[k][:, m * 128 : (m + 1) * 128],
                            xr[k][:],
                            start=(k == 0),
                            stop=(k == 2),
                        )
                    tht = th_pool.tile([128, NT], bf, tag="tht")
                    nc.scalar.activation(tht[:], ps[:], AF.Tanh, bias=bxh_t[:, m : m + 1])
                    nc.sync.dma_start(th_d[m][:, n * NT : (n + 1) * NT], tht[:])

            # G0 = W_ihA @ t_h + (b_ih + b_hh)   (bf16, tiled [r][n][128, 4*NT])
            wa_pool = pro.enter_context(tc.tile_pool(name="pwa", bufs=1))
            wa = [wa_pool.tile([128, G4], bf, tag=f"wa{k}", name=f"wa{k}") for k in range(8)]
            for k in range(8):
                nc.sync.dma_start(wa[k][:], WihAT[k])
            thr_pool = pro.enter_context(tc.tile_pool(name="pthr", bufs=2))
            g0_pool = pro.enter_context(tc.tile_pool(name="pg0", bufs=2))
            for n in range(NB):
                thr = [thr_pool.tile([128, NT], bf, tag=f"thr{k}", name=f"thr{k}") for k in range(8)]
                for k in range(8):
                    nc.sync.dma_start(thr[k][:], th_d[k][:, n * NT : (n + 1) * NT])
                for r in range(8):
                    g0t = g0_pool.tile([128, 4 * NT], bf, tag="g0t")
                    for gi in range(4):
                        m = gi * 8 + r
                        ps = pspool.tile([128, NT], f32, tag="ps")
                        for k in range(8):
                            nc.tensor.matmul(
                                ps[:],
                                wa[k][:, m * 128 : (m + 1) * 128],
                                thr[k][:],
                                start=(k == 0),
                                stop=(k == 7),
                            )
                        nc.scalar.activation(
                            g0t[:, gi * NT : (gi + 1) * NT],
                            ps[:],
                            AF.Identity,
                            bias=bg_t[:, m : m + 1],
                        )
                    nc.sync.dma_start(G0_d[r, n], g0t[:])

        # ================= resident weights =================
        wres = ctx.enter_context(tc.tile_pool(name="wres", bufs=1))
        whp = [wres.tile([128, 2, G4], f8, tag=f"whp{k}", name=f"whp{k}") for k in range(4)]
        for k in range(4):
            nc.sync.dma_start(whp[k][:], WhhP[k])
        wb_t = wres.tile([A, G4], bf, tag="wbig")
        nc.sync.dma_start(wb_t[:], WbigT[:])
        wzp = [wres.tile([128, 2, A], f8, tag=f"wzp{k}", name=f"wzp{k}") for k in range(4)]
        for k in range(4):
            nc.sync.dma_start(wzp[k][:], WhzP[k])
        bhz_t = wres.tile([A, 1], f32, tag="bhz")
        nc.sync.dma_start(bhz_t[:], bhz[:])
        # all-ones [A, A] lhsT: one matmul = column-sum broadcast to A partitions
        onesbc = wres.tile([A, A], bf, tag="onesbc")
        nc.vector.memset(onesbc[:], 1.0)
        # identity lhsT: PE-injects G0 into PSUM as the accumulation base
        eye_t = wres.tile([128, 128], bf, tag="eye")
        nc.sync.dma_start(eye_t[:], eyeT[:])

        # ================= main loop pools =================
        # PSUM budget (8 banks): 3 x 2-bank gate-pair ring + zps + sb
        psg = ctx.enter_context(tc.tile_pool(name="psg", bufs=3, space="PSUM"))
        psz = ctx.enter_context(tc.tile_pool(name="psz", bufs=1, space="PSUM"))
        g0r_p = ctx.enter_context(tc.tile_pool(name="g0r", bufs=1))
        h_p = ctx.enter_context(tc.tile_pool(name="hp", bufs=2))
        c_p = ctx.enter_context(tc.tile_pool(name="cp", bufs=1))
        gt_p = ctx.enter_context(tc.tile_pool(name="gt", bufs=2))
        cw_p = ctx.enter_context(tc.tile_pool(name="cw", bufs=2))
        z_p = ctx.enter_context(tc.tile_pool(name="zp", bufs=2))
        pp_p = ctx.enter_context(tc.tile_pool(name="pp", bufs=2))

        def block_body(nb_iv):
            """All nsteps LSTM steps for one 512-column batch block."""
            g0res = [
                g0r_p.tile([128, 4 * NT], bf, tag=f"g0r{r}", name=f"g0r{r}")
                for r in range(8)
            ]
            for r in range(8):
                if isinstance(nb_iv, int):
                    nc.sync.dma_start(g0res[r][:], G0_d[r, nb_iv])
                else:
                    nc.sync.dma_start(g0res[r][:], G0_d[r][bass.ds(nb_iv, 1)])

            ctiles = [None] * 8

            def z_phase(t, hcur):
                """p = softplus(z)/sum(softplus(z)) with z = Whz@h + bhz.
                softplus(z) = -ln(sigmoid(-z)); the negation cancels in the
                normalization, so p = ln(sigmoid(-z)) / sum(ln(sigmoid(-z)))
                exactly. sigmoid stays in the gate act-table set; only ln
                switches tables."""
                zps = psz.tile([A, NT], f32, tag="zps", name="zps")
                for kk in range(4):
                    nc.tensor.matmul(
                        zps[:],
                        wzp[kk][:],
                        hcur[kk][:],
                        start=(kk == 0),
                        stop=(kk == 3),
                        perf_mode=DR,
                    )
                # bhz_t holds -b_hz (host-negated): sigmoid(-z) = sig(zps*-1 + -bhz)
                u = z_p.tile([A, NT], f32, tag="u")
                nc.scalar.activation(u[:], zps[:], AF.Sigmoid, bias=bhz_t[:], scale=-1.0)
                q2 = z_p.tile([A, NT], bf, tag="q2")
                nc.scalar.activation(q2[:], u[:], AF.Ln)
                sb = psz.tile([A, NT], f32, tag="sb", name="sb")
                nc.tensor.matmul(sb[:], onesbc[:], q2[:], start=True, stop=True)
                rec = z_p.tile([A, NT], f32, tag="rec")
                nc.vector.reciprocal(rec[:], sb[:])
                pnew = pp_p.tile([A, NT], bf, tag="p", name="pnew")
                nc.vector.tensor_tensor(pnew[:], q2[:], rec[:], mybir.AluOpType.mult)
                if isinstance(nb_iv, int):
                    nc.sync.dma_start(p_out[t, nb_iv], pnew[:])
                else:
                    nc.sync.dma_start(p_out[t][bass.ds(nb_iv, 1)], pnew[:])
                return pnew

            def cell_and_h(r, i_sb, f_sb, g_sb, to_sb, hcur, t):
                """c' = f*c + i*g (Pool); h~ = (tanh(x_o/2)+1)*tanh(c') -> fp8.

                h~ = 2h; the 2x is folded into 0.5-scaled W_hh / W_hz host-side,
                and x_o/2 into 0.5-scaled o-gate weight rows."""
                if t == 0:
                    c_r = c_p.tile([128, NT], f32, tag=f"c{r}", name=f"c{r}")
                    nc.gpsimd.tensor_tensor(c_r[:], g_sb, i_sb, mybir.AluOpType.mult)
                    ctiles[r] = c_r
                else:
                    ig = cw_p.tile([128, NT], f32, tag="ig")
                    nc.gpsimd.tensor_tensor(ig[:], g_sb, i_sb, mybir.AluOpType.mult)
                    nc.gpsimd.tensor_tensor(f_sb, f_sb, ctiles[r][:], mybir.AluOpType.mult)
                    nc.gpsimd.tensor_tensor(ctiles[r][:], f_sb, ig[:], mybir.AluOpType.add)
                tht = cw_p.tile([128, NT], f32, tag="tht")
                nc.scalar.activation(tht[:], ctiles[r][:], AF.Tanh)
                kk, j = r // 2, r % 2
                if j == 0:
                    hp_t = h_p.tile([128, 2, NT], DT.float8e4, tag=f"h{kk}", name=f"h{kk}")
                    hcur.append(hp_t)
                nc.vector.scalar_tensor_tensor(
                    hcur[kk][:, j], to_sb, 1.0, tht[:],
                    mybir.AluOpType.add, mybir.AluOpType.mult,
                )

            # ---- step 0: h = c = p = 0, so gates = act(G0) straight from SBUF
            hcur = []
            for r in range(8):
                i_sb = gt_p.tile([128, NT], f32, tag="gif", name="i_sb")
                nc.scalar.activation(i_sb[:], g0res[r][:, 0:NT], AF.Sigmoid)
                go_sb = gt_p.tile([128, 2 * NT], f32, tag="ggo", name="go_sb")
                nc.scalar.activation(go_sb[:], g0res[r][:, 2 * NT : 4 * NT], AF.Tanh)
                cell_and_h(
                    r, i_sb[:], None, go_sb[:, 0:NT], go_sb[:, NT : 2 * NT], hcur, 0
                )
            pprev = z_phase(0, hcur)
            hprev = hcur

            # ---- steps 1..nsteps-1
            for t in range(1, nsteps):
                hcur = []
                for r in range(8):
                    # (i, f) pair in one 2-bank PSUM tile; (g, o) likewise
                    pif = psg.tile([128, 2 * NT], f32, tag="ps2", name="pif")
                    pgo = psg.tile([128, 2 * NT], f32, tag="ps2", name="pgo")
                    for gi in range(4):
                        m = gi * 8 + r
                        ps = (pif if gi < 2 else pgo)[:, (gi % 2) * NT : (gi % 2 + 1) * NT]
                        # seed PSUM with G0 via identity matmul, accumulate gates
                        nc.tensor.matmul(
                            ps,
                            eye_t[:],
                            g0res[r][:, gi * NT : (gi + 1) * NT],
                            start=True,
                            stop=False,
                        )
                        for kk in range(4):
                            nc.tensor.matmul(
                                ps,
                                whp[kk][:, :, m * 128 : (m + 1) * 128],
                                hprev[kk][:],
                                start=False,
                                stop=False,
                                perf_mode=DR,
                            )
                        nc.tensor.matmul(
                            ps,
                            wb_t[:, m * 128 : (m + 1) * 128],
                            pprev[:],
                            start=False,
                            stop=True,
                        )
                    if_sb = gt_p.tile([128, 2 * NT], f32, tag="gif", name="if_sb")
                    nc.scalar.activation(if_sb[:], pif[:], AF.Sigmoid)
                    go_sb = gt_p.tile([128, 2 * NT], f32, tag="ggo", name="go_sb")
                    nc.scalar.activation(go_sb[:], pgo[:], AF.Tanh)
                    cell_and_h(
                        r, if_sb[:, 0:NT], if_sb[:, NT : 2 * NT],
                        go_sb[:, 0:NT], go_sb[:, NT : 2 * NT],
                        hcur, t,
                    )
                pprev = z_phase(t, hcur)
                hprev = hcur

        if use_for_i:
            with tc.For_i(
                0,
                nblocks,
                1,
                hint_engines=(ET.PE, ET.Activation, ET.DVE, ET.Pool),
            ) as nb_iv:
                block_body(nb_iv)
        else:
            for nbi in range(nblocks):
                block_body(nbi)

    nc.compile()
    return nc


# ---------------- host-side wrapper ----------------


def _prep_weights(W_xh, b_xh, W_ih, W_hh, b_ih, b_hh, W_hz, b_hz, W_emb):
    bf = ml_dtypes.bfloat16
    f8 = ml_dtypes.float8_e4m3
    f32 = np.float32
    d = {}
    wxh = np.zeros((KXP, H), f32)
    wxh[:E] = np.asarray(W_xh, f32).T
    d["WxhT"] = np.ascontiguousarray(wxh.reshape(3, 128, H)).astype(bf)
    d["bxh"] = np.ascontiguousarray(np.asarray(b_xh, f32).reshape(8, 128).T)
    # o-gate rows (3H:4H) pre-scaled by 0.5: kernel computes tanh(x_o/2) and
    # reconstructs 2*sigmoid(x_o) = tanh(x_o/2)+1 (the 2x is h~ = 2h).
    wih = np.asarray(W_ih, f32).copy()
    wih[3 * H :] *= 0.5
    d["WihAT"] = np.ascontiguousarray(wih[:, :H].T.reshape(8, 128, G4)).astype(bf)
    wbig = wih[:, H:].astype(np.float64) @ np.asarray(W_emb, np.float64)
    d["WbigT"] = np.ascontiguousarray(wbig.T.astype(np.float32)).astype(bf)
    # W_hh, W_hz scaled 0.5 to absorb h~ = 2h; o-rows of W_hh a further 0.5.
    whh = np.asarray(W_hh, f32) * 0.5
    whh[3 * H :] *= 0.5
    # k-paired fp8 layouts for DoubleRow: [kk, p, j, m] = W[m, kk*256+j*128+p]
    whT = whh.T.reshape(4, 2, 128, G4).transpose(0, 2, 1, 3)
    d["WhhP"] = np.ascontiguousarray(whT).astype(f8)
    bgv = (np.asarray(b_ih, f32) + np.asarray(b_hh, f32)).copy()
    bgv[3 * H :] *= 0.5
    d["bg"] = np.ascontiguousarray(bgv.reshape(32, 128).T)
    wzT = (np.asarray(W_hz, f32) * 0.5).T.reshape(4, 2, 128, A).transpose(0, 2, 1, 3)
    d["WhzP"] = np.ascontiguousarray(wzT).astype(f8)
    # negated: kernel computes sigmoid(-z) = sig(zps*-1 + (-bhz))
    d["bhz"] = np.ascontiguousarray(-np.asarray(b_hz, f32).reshape(A, 1))
    d["eyeT"] = np.eye(128, dtype=f32).astype(bf)
    return d


def _prep_x(x_shard):
    bf = ml_dtypes.bfloat16
    xt = np.zeros((KXP, x_shard.shape[0]), np.float32)
    xt[:E] = np.asarray(x_shard, np.float32).T
    return np.ascontiguousarray(xt.reshape(3, 128, -1)).astype(bf)


def kernel(input_x, W_xh, b_xh, W_ih, W_hh, b_ih, b_hh, W_hz, b_hz, W_emb):
    from concourse.bass_utils import run_bass_kernel_spmd

    wd = _prep_weights(W_xh, b_xh, W_ih, W_hh, b_ih, b_hh, W_hz, b_hz, W_emb)
    x = np.asarray(input_x, np.float32)
    in_maps = []
    for c in range(NCORES):
        m = dict(wd)
        m["xT"] = _prep_x(x[c * BL : (c + 1) * BL])
        in_maps.append(m)

    nc = build_nc()
    res = run_bass_kernel_spmd(nc, in_maps, list(range(NCORES)))
    global LAST_RESULT
    LAST_RESULT = res

    out = np.empty((B, D, A), np.float32)
    for c in range(NCORES):
        pa = np.asarray(res.results[c]["p_out"], dtype=np.float32)  # [D, NB, A, NT]
        # [D, NB, A, NT] -> [NB, NT, D, A] -> [BL, D, A]
        out[c * BL : (c + 1) * BL] = pa.transpose(1, 3, 0, 2).reshape(BL, D, A)
    return out, out
